# revision 12
# baseline (speedup 1.0000x reference)
# ChebConv (K=4) + BatchNorm + LeakyReLU, distributed over 8 TRN2 NeuronCores.
#
# s-space formulation: track s_k = dinv * T_k, so the gather tables need no
# per-round rescale and the recurrence is
#   s_1 = nd1 * seg,   s_k = nd2 * seg - s_{k-2},   nd1 = -dinv^2, nd2 = -2dinv^2
# with seg[dst] = sum_{e->dst} s_{k-1}[src]. dinv/sqrt(deg)/nd1/nd2 come from
# the host (no on-device degree pass). Projection applies T_k = sqrt(deg)*s_k
# per window (ACT), transposes on the PE and accumulates outT += W_k^T T_k^T.
#
# Precision/speed: tables are stored as fp16 hi/lo pairs interleaved per row
# ([hi(256B) | lo(256B)] = 512B rows, same bytes as fp32, ~2^-22 relative
# accuracy). One dma_gather per edge fetches both halves; the segment sum is
# two fp16 matmuls (one-hot S is exact in fp16) accumulating into the same
# fp32 PSUM, so the PE runs at 16-bit speed with fp32-class accuracy. All
# remaining state (A/B, outT, psum, post-scales) is fp32, matching the fp32
# baseline's error profile.
#
# Nodes split into 8 shards; edges partitioned by destination core, grouped by
# (src chunk, dst window of 128); per-edge source rows gathered from per-chunk
# AllGathered tables; BN stats via a tiny AllReduce; bias b cancels in BN.

import numpy as np

from concourse import bass, bacc, mybir
import concourse.tile as tile
from concourse.masks import make_identity
from concourse.library_config import mlp as mlp_lib

P = 128
F = 128
FP32 = mybir.dt.float32
FP16 = mybir.dt.float16
I16 = mybir.dt.int16
I32 = mybir.dt.int32
AOp = mybir.AluOpType
AF = mybir.ActivationFunctionType
AX = mybir.AxisListType
BN_EPS = 1e-5
LEAKY = 0.01
MAXG = 1024  # dma_gather breaks above 1024 idxs per instruction


def _cdiv(a, b):
    return -(-a // b)


def plan(edge_idx, N, M, nch=5):
    """Host-side layout prep: edge partitioning/sorting + packed index arrays."""
    row = np.asarray(edge_idx[0], dtype=np.int64)
    col = np.asarray(edge_idx[1], dtype=np.int64)
    shard = N // M
    assert shard * M == N
    W = _cdiv(shard, P)
    win_rows = [min(P, shard - w * P) for w in range(W)]

    deg = np.bincount(row, minlength=N).astype(np.float64)
    assert deg.min() > 0, "deg-0 nodes unsupported by s-space kernel"
    dinv = 1.0 / np.sqrt(deg)
    sqrtdeg = np.sqrt(deg)

    base, rem = W // nch, W % nch
    ch_nw = [base + (1 if c < rem else 0) for c in range(nch)]
    ch_w0 = np.cumsum([0] + ch_nw)[:-1].tolist()
    ch_r0 = [min(w0 * P, shard) for w0 in ch_w0]
    ch_rows = []
    for c in range(nch):
        r1 = min((ch_w0[c] + ch_nw[c]) * P, shard)
        ch_rows.append(r1 - ch_r0[c])
    assert all(M * r <= 32767 for r in ch_rows), (M, ch_rows)

    dst_core = col // shard
    dloc = col % shard
    win = dloc // P
    src_core = row // shard
    sloc = row % shard
    ch_bounds = np.array(ch_r0 + [shard], dtype=np.int64)
    src_ch = np.searchsorted(ch_bounds, sloc, side="right") - 1
    idx16 = src_core * np.asarray(ch_rows, dtype=np.int64)[src_ch] + (
        sloc - ch_bounds[src_ch]
    )

    # groups keyed (dst_core, src_ch, win)
    gkey = (dst_core * nch + src_ch) * W + win
    counts = np.bincount(gkey, minlength=M * nch * W).reshape(M, nch, W)
    caps = np.zeros((nch, W), dtype=np.int64)
    for c in range(nch):
        for w in range(W):
            mx = counts[:, c, w].max()
            caps[c][w] = _cdiv(mx, P) * P if mx > 0 else 0

    off_pad = np.zeros((nch, W), dtype=np.int64)
    t = 0
    for c in range(nch):
        for w in range(W):
            off_pad[c][w] = t
            t += caps[c][w]
    tot_pad = t

    order = np.lexsort((gkey, dst_core))
    idx16_arrs, colloc_arrs = [], []
    dinv_arrs, sqd_arrs, nd1_arrs, nd2_arrs = [], [], [], []
    col_in_win = dloc % P
    for m in range(M):
        sel = order[dst_core[order] == m]
        k = gkey[sel] % (nch * W)  # (c, w) flat index
        ks = np.argsort(k, kind="stable")
        sel = sel[ks]
        k = k[ks]
        grp_start = np.searchsorted(k, np.arange(nch * W))
        j = np.arange(sel.size) - grp_start[k]
        pos = off_pad.reshape(-1)[k] + j
        idx_flat = np.zeros(tot_pad, dtype=np.int16)  # pad idx 0 = valid row
        cl_flat = np.full(tot_pad, -1.0, dtype=np.float32)
        idx_flat[pos] = idx16[sel].astype(np.int16)
        cl_flat[pos] = col_in_win[sel].astype(np.float32)
        ia = np.zeros((16, tot_pad // 16), dtype=np.int16)
        ia[pos % 16, pos // 16] = idx_flat[pos]
        idx16_arrs.append(np.tile(ia, (8, 1)))
        ca = np.full((P, tot_pad // P), -1.0, dtype=np.float32)
        ca[pos % P, pos // P] = cl_flat[pos]
        colloc_arrs.append(ca)

        ids = np.arange(shard)
        gl = dinv[m * shard + ids]
        sq = sqrtdeg[m * shard + ids]

        def fold(v):
            a = np.zeros((P, W), dtype=np.float32)
            a[ids % P, ids // P] = v
            return a

        dinv_arrs.append(fold(gl))
        sqd_arrs.append(fold(sq))
        nd1_arrs.append(fold(-gl * gl))
        nd2_arrs.append(fold(-2.0 * gl * gl))

    return dict(
        N=N, M=M, shard=shard, W=W, win_rows=win_rows, nch=nch,
        ch_nw=ch_nw, ch_w0=ch_w0, ch_r0=ch_r0, ch_rows=ch_rows,
        caps=caps, off_pad=off_pad, tot_pad=tot_pad,
        idx16_arrs=idx16_arrs, colloc_arrs=colloc_arrs,
        dinv_arrs=dinv_arrs, sqd_arrs=sqd_arrs,
        nd1_arrs=nd1_arrs, nd2_arrs=nd2_arrs,
    )


def which_chunk(ch_w0, ch_nw, w):
    for c in range(len(ch_w0)):
        if ch_w0[c] <= w < ch_w0[c] + ch_nw[c]:
            return c
    raise AssertionError


def build(nc, cfg, K, no_cc=False):
    M, shard, W, nch = cfg["M"], cfg["shard"], cfg["W"], cfg["nch"]
    win_rows, caps, off_pad = cfg["win_rows"], cfg["caps"], cfg["off_pad"]
    ch_nw, ch_w0, ch_r0, ch_rows = (
        cfg["ch_nw"], cfg["ch_w0"], cfg["ch_r0"], cfg["ch_rows"],
    )
    N = cfg["N"]
    rg = [list(range(M))]
    shared_as = "Shared" if M > 4 else "Local"
    capmax = int(max(caps.max(), 1))
    F2 = 2 * F

    x_d = nc.dram_tensor("x_sh", [shard, F], FP32, kind="ExternalInput").ap()
    w_d = nc.dram_tensor("w_all", [K, F, F], FP32, kind="ExternalInput").ap()
    gam_d = nc.dram_tensor("gamma", [F, 1], FP32, kind="ExternalInput").ap()
    bet_d = nc.dram_tensor("beta", [F, 1], FP32, kind="ExternalInput").ap()
    idx_d = nc.dram_tensor(
        "idx16", [P, cfg["tot_pad"] // 16], I16, kind="ExternalInput"
    ).ap()
    cl_d = nc.dram_tensor(
        "colloc", [P, cfg["tot_pad"] // P], FP32, kind="ExternalInput"
    ).ap()
    nd_d = nc.dram_tensor("dinv_w", [P, W], FP32, kind="ExternalInput").ap()
    sq_d = nc.dram_tensor("sqd_w", [P, W], FP32, kind="ExternalInput").ap()
    n1_d = nc.dram_tensor("nd1_w", [P, W], FP32, kind="ExternalInput").ap()
    n2_d = nc.dram_tensor("nd2_w", [P, W], FP32, kind="ExternalInput").ap()
    out_d = nc.dram_tensor("out_t", [P, shard], FP32, kind="ExternalOutput").ap()

    with tile.TileContext(nc) as tc:
        with (
            tc.tile_pool(name="persist", bufs=1) as pp,
            tc.tile_pool(name="stage", bufs=3) as sp,
            tc.tile_pool(name="idxp", bufs=2) as ip,
            tc.tile_pool(name="clp", bufs=2) as cp,
            tc.tile_pool(name="hilo", bufs=1) as hp,
            tc.tile_pool(name="gath", bufs=2) as gp,
            tc.tile_pool(name="sbuild", bufs=4) as sbp,
            tc.tile_pool(name="vec", bufs=4) as vp,
            tc.tile_pool(name="roll", bufs=2) as rp,
            tc.tile_pool(name="ps_g", bufs=3, space="PSUM") as pg,
            tc.tile_pool(name="ps_sm", bufs=2, space="PSUM") as psm,
            tc.tile_pool(name="ps_o", bufs=2, space="PSUM") as po,
            tc.tile_pool(name="dram", bufs=1, space="DRAM") as dp,
        ):
            # ---- persistent SBUF
            A = pp.tile([P, W * F], FP32, name="Abuf")
            B = pp.tile([P, W * F], FP32, name="Bbuf")
            outT = pp.tile([P, shard], FP32, name="outT")
            ident = pp.tile([P, P], FP32, name="ident")
            iota_i = pp.tile([P, P], I32, name="iota_i")
            iota_f = pp.tile([P, P], FP32, name="iota_f")
            W_sb = pp.tile([P, K * F], FP32, name="W_sb")
            gam = pp.tile([P, 1], FP32, name="gam")
            bet = pp.tile([P, 1], FP32, name="bet")
            dinv = pp.tile([P, W], FP32, name="dinv")
            sqd = pp.tile([P, W], FP32, name="sqd")
            nd1 = pp.tile([P, W], FP32, name="nd1")
            nd2 = pp.tile([P, W], FP32, name="nd2")
            eps_t = pp.tile([P, 1], FP32, name="eps_t")

            make_identity(nc, ident[:])
            nc.gpsimd.iota(iota_i[:], pattern=[[1, P]], base=0, channel_multiplier=0)
            nc.gpsimd.load_library(mlp_lib)
            nc.vector.tensor_copy(iota_f[:], iota_i[:])
            nc.vector.memset(eps_t[:], BN_EPS)
            for k in range(K):
                nc.sync.dma_start(W_sb[:, k * F:(k + 1) * F], w_d[k])
            nc.sync.dma_start(gam[:], gam_d[:])
            nc.sync.dma_start(bet[:], bet_d[:])
            nc.sync.dma_start(dinv[:], nd_d[:])
            nc.sync.dma_start(sqd[:], sq_d[:])
            nc.sync.dma_start(nd1[:], n1_d[:])
            nc.sync.dma_start(nd2[:], n2_d[:])

            cap_regs = {}

            def cap_reg(cap):
                if cap not in cap_regs:
                    cap_regs[cap] = nc.gpsimd.to_reg(cap)
                return cap_regs[cap]

            def wslice(buf, w):
                return buf[:, w * F:(w + 1) * F]

            u_in = [
                dp.tile([ch_rows[c], F2], FP16, name=f"u_in{c}")
                for c in range(nch)
            ]
            u_out = [
                [
                    dp.tile(
                        [M * ch_rows[c], F2], FP16,
                        name=f"u_out{c}_{kr}", addr_space=shared_as,
                    )
                    for kr in range(K - 1)
                ]
                for c in range(nch)
            ]
            bn_in = dp.tile([P, 2], FP32, name="bn_in")
            bn_out = dp.tile([P, 2], FP32, name="bn_out", addr_space=shared_as)

            # ---- W_k projection: outT += W_k^T @ (sqd * s_k)^T per 4-window
            def wk_chain(src_buf, k, w, troll_box):
                q, pos = w // 4, w % 4
                if pos == 0:
                    troll_box[0] = rp.tile([P, 4 * P], FP32, name="troll")
                troll = troll_box[0]
                tk = sp.tile([P, F], FP32, name="tk")
                nc.scalar.activation(
                    tk[:], wslice(src_buf, w), AF.Identity,
                    scale=sqd[:, w:w + 1],
                )
                ps_t = psm.tile([P, P], FP32, name="ps_small", tag="ps_small")
                nc.tensor.transpose(ps_t[:], tk[:], ident[:])
                rw = win_rows[w]
                nc.scalar.copy(troll[:, pos * P: pos * P + rw], ps_t[:, :rw])
                if w == min(4 * q + 3, W - 1):
                    node0 = q * 4 * P
                    ncols = min(4 * P, shard - node0)
                    ps_o = po.tile([P, 4 * P], FP32, name="ps_o")
                    nc.tensor.matmul(
                        ps_o[:, :ncols],
                        lhsT=W_sb[:, k * F:(k + 1) * F],
                        rhs=troll[:, :ncols],
                        start=True, stop=True,
                    )
                    sl = outT[:, node0: node0 + ncols]
                    if k == 0:
                        nc.vector.tensor_copy(sl, ps_o[:, :ncols])
                    else:
                        nc.vector.tensor_tensor(
                            sl, sl, ps_o[:, :ncols], op=AOp.add
                        )

            def u_write(src_buf, c2, kround):
                # split chunk slab into fp16 hi/lo and store interleaved rows
                nw = ch_nw[c2]
                w0 = ch_w0[c2]
                sl = src_buf[:, w0 * F:(w0 + nw) * F]
                hi = hp.tile([P, nw * F], FP16, name="hi_sl")
                lo = hp.tile([P, nw * F], FP16, name="lo_sl")
                nc.scalar.copy(hi[:], sl)
                nc.vector.tensor_tensor(lo[:], sl, hi[:], op=AOp.subtract)
                nw_full = nw if win_rows[w0 + nw - 1] == P else nw - 1
                for half, buf in ((0, hi), (1, lo)):
                    if nw_full:
                        nc.sync.dma_start(
                            u_in[c2][: nw_full * P, :].rearrange(
                                "(w p) f -> p w f", p=P
                            )[:, :, half * F:(half + 1) * F],
                            buf[:].rearrange("p (w f) -> p w f", f=F)[
                                :, :nw_full, :
                            ],
                        )
                    rem = win_rows[w0 + nw - 1]
                    if rem < P:
                        nc.sync.dma_start(
                            u_in[c2][
                                nw_full * P:, half * F:(half + 1) * F
                            ],
                            buf[:rem, (nw - 1) * F: nw * F],
                        )
                if not no_cc:
                    nc.gpsimd.collective_compute(
                        "AllGather", AOp.bypass, replica_groups=rg,
                        ins=[u_in[c2].opt()], outs=[u_out[c2][kround].opt()],
                    )

            # ========== round 0: load x, s0 = dinv*x, W0 proj, AG ==========
            troll_box = [None]
            for w in range(W):
                rw = win_rows[w]
                xt = sp.tile([P, F], FP32, name="xt")
                nc.sync.dma_start(xt[:rw, :], x_d[w * P: w * P + rw, :])
                if rw < P:
                    nc.vector.memset(wslice(A, w), 0.0)
                nc.scalar.activation(
                    wslice(A, w)[:rw, :], xt[:rw, :], AF.Identity,
                    scale=dinv[:rw, w:w + 1],
                )
                wk_chain(A, 0, w, troll_box)
                c2 = which_chunk(ch_w0, ch_nw, w)
                if w == ch_w0[c2] + ch_nw[c2] - 1:
                    u_write(A, c2, 0)

            # ========== rounds 1..K-1 ====================================
            ch_off = [int(off_pad[c][0]) for c in range(nch)]
            ch_len = [
                (int(off_pad[c + 1][0]) if c + 1 < nch else cfg["tot_pad"])
                - ch_off[c]
                for c in range(nch)
            ]
            for k in range(1, K):
                dst = B if k % 2 == 1 else A
                nd = nd1 if k == 1 else nd2
                troll_box = [None]
                for c in range(nch):
                    tab = u_out[c][k - 1]
                    wmid = W // 2
                    h_off = [ch_off[c], int(off_pad[c][wmid])]
                    h_end = [h_off[1], ch_off[c] + ch_len[c]]
                    hmax = max(
                        max(
                            int(off_pad[cc][wmid]) - co,
                            co + cl - int(off_pad[cc][wmid]),
                        )
                        for cc, (co, cl) in enumerate(zip(ch_off, ch_len))
                    )
                    hmax = _cdiv(hmax, 2048) * 2048
                    it_h = [None, None]
                    for h in (0, 1):
                        ln = h_end[h] - h_off[h]
                        if h == 1 and ln == 0:
                            continue
                        it_h[h] = ip.tile([P, hmax // 16], I16, name="it_c")
                        nc.sync.dma_start(
                            it_h[h][:, : ln // 16],
                            idx_d[:, h_off[h] // 16: h_end[h] // 16],
                        )
                    cl_c = cp.tile([P, max(ch_len) // P], FP32, name="cl_c")
                    nc.sync.dma_start(
                        cl_c[:, : ch_len[c] // P],
                        cl_d[:, ch_off[c] // P: (ch_off[c] + ch_len[c]) // P],
                    )
                    for w in range(W):
                        cap = int(caps[c][w])
                        first = all(caps[cc][w] == 0 for cc in range(c))
                        if cap > 0:
                            g = cap // P
                            h = 0 if w < wmid else 1
                            it_c = it_h[h]
                            o16 = (int(off_pad[c][w]) - h_off[h]) // 16
                            ot = (int(off_pad[c][w]) - ch_off[c]) // P
                            U = gp.tile([P, capmax * 2], FP16, name="Ug")
                            Uv = U[:].rearrange("p (g f) -> p g f", f=F2)
                            for goff in range(0, cap, MAXG):
                                sub = min(MAXG, cap - goff)
                                nc.gpsimd.dma_gather(
                                    out_ap=Uv[:, goff // P:(goff + sub) // P, :],
                                    in_ap=tab[:],
                                    idxs_ap=it_c[
                                        :, o16 + goff // 16: o16 + (goff + sub) // 16
                                    ],
                                    num_idxs=sub,
                                    num_idxs_reg=cap_reg(sub),
                                    elem_size=F2,
                                )
                            ps_g = pg.tile([P, P], FP32, name="ps_g")
                            for t in range(g):
                                S = sbp.tile([P, P], FP16, name="Sb")
                                nc.vector.tensor_scalar(
                                    out=S[:], in0=iota_f[:],
                                    scalar1=cl_c[:, ot + t: ot + t + 1],
                                    scalar2=None, op0=AOp.is_equal,
                                )
                                nc.tensor.matmul(
                                    ps_g[:], lhsT=S[:], rhs=Uv[:, t, 0:F],
                                    start=(t == 0), stop=False,
                                )
                                nc.tensor.matmul(
                                    ps_g[:], lhsT=S[:], rhs=Uv[:, t, F:F2],
                                    start=False, stop=(t == g - 1),
                                )
                            dw = wslice(dst, w)
                            if first and k == 1:
                                nc.vector.tensor_scalar_mul(
                                    out=dw, in0=ps_g[:],
                                    scalar1=nd[:, w:w + 1],
                                )
                            else:
                                nc.vector.scalar_tensor_tensor(
                                    out=dw, in0=ps_g[:],
                                    scalar=nd[:, w:w + 1],
                                    in1=dw, op0=AOp.mult,
                                    op1=(AOp.subtract if first else AOp.add),
                                )
                        elif first and c == nch - 1 and all(
                            caps[cc][w] == 0 for cc in range(nch)
                        ):
                            dw = wslice(dst, w)
                            if k == 1:
                                nc.vector.memset(dw, 0.0)
                            else:
                                nc.vector.tensor_scalar_mul(
                                    out=dw, in0=dw, scalar1=-1.0
                                )
                        if c == nch - 1:
                            wk_chain(dst, k, w, troll_box)
                            c2 = which_chunk(ch_w0, ch_nw, w)
                            if k < K - 1 and w == ch_w0[c2] + ch_nw[c2] - 1:
                                u_write(dst, c2, k)

            # ========== BatchNorm + LeakyReLU ============================
            s1 = vp.tile([P, 1], FP32, name="s1")
            nc.vector.reduce_sum(out=s1[:], in_=outT[:, :shard], axis=AX.X)
            s2 = pp.tile([P, 1], FP32, name="s2acc")
            nc.vector.memset(s2[:], 0.0)
            CH = 4 * P
            for n0 in range(0, shard, CH):
                n1 = min(n0 + CH, shard)
                sqs = rp.tile([P, 4 * P], FP32, name="troll")
                s2p = vp.tile([P, 1], FP32, name="s2p")
                nc.scalar.activation(
                    sqs[:, : n1 - n0], outT[:, n0:n1], AF.Square,
                    accum_out=s2p[:],
                )
                nc.vector.tensor_tensor(s2[:], s2[:], s2p[:], op=AOp.add)
            bn_sb = pp.tile([P, 2], FP32, name="bn_sb")
            nc.vector.tensor_copy(bn_sb[:, 0:1], s1[:])
            nc.vector.tensor_copy(bn_sb[:, 1:2], s2[:])
            nc.sync.dma_start(bn_in[:], bn_sb[:])
            if not no_cc:
                nc.gpsimd.collective_compute(
                    "AllReduce", AOp.add, replica_groups=rg,
                    ins=[bn_in.opt()], outs=[bn_out.opt()],
                )
            bnr = pp.tile([P, 2], FP32, name="bnr")
            nc.sync.dma_start(bnr[:], bn_out[:])
            mean = vp.tile([P, 1], FP32, name="mean")
            msq = vp.tile([P, 1], FP32, name="msq")
            nc.scalar.mul(mean[:], bnr[:, 0:1], 1.0 / N)
            nc.scalar.mul(msq[:], bnr[:, 1:2], 1.0 / N)
            m2 = vp.tile([P, 1], FP32, name="m2")
            var = vp.tile([P, 1], FP32, name="var")
            nc.vector.tensor_tensor(m2[:], mean[:], mean[:], op=AOp.mult)
            nc.vector.tensor_tensor(var[:], msq[:], m2[:], op=AOp.subtract)
            stdv = pp.tile([P, 1], FP32, name="stdv")
            rstd = pp.tile([P, 1], FP32, name="rstd")
            nc.scalar.activation(stdv[:], var[:], AF.Sqrt, bias=eps_t[:])
            nc.vector.reciprocal(rstd[:], stdv[:])
            Aaff = pp.tile([P, 1], FP32, name="Aaff")
            Baff = pp.tile([P, 1], FP32, name="Baff")
            mA = vp.tile([P, 1], FP32, name="mA")
            nc.vector.tensor_tensor(Aaff[:], gam[:], rstd[:], op=AOp.mult)
            nc.vector.tensor_tensor(mA[:], mean[:], Aaff[:], op=AOp.mult)
            nc.vector.tensor_tensor(Baff[:], bet[:], mA[:], op=AOp.subtract)
            for n0 in range(0, shard, CH):
                n1 = min(n0 + CH, shard)
                ts = rp.tile([P, 4 * P], FP32, name="troll")
                nc.scalar.activation(
                    ts[:, : n1 - n0], outT[:, n0:n1], AF.Identity,
                    bias=Baff[:], scale=Aaff[:],
                )
                nc.vector.scalar_tensor_tensor(
                    out=outT[:, n0:n1], in0=ts[:, : n1 - n0], scalar=LEAKY,
                    in1=ts[:, : n1 - n0], op0=AOp.mult, op1=AOp.max,
                )
            nc.sync.dma_start(out_d[:], outT[:, :shard])
    return nc


def make_in_maps(cfg, x, W_, gamma, beta):
    M, shard = cfg["M"], cfg["shard"]
    x = np.asarray(x, dtype=np.float32)
    maps = []
    for m in range(M):
        maps.append(
            {
                "x_sh": np.ascontiguousarray(x[m * shard:(m + 1) * shard]),
                "w_all": np.asarray(W_, dtype=np.float32),
                "gamma": np.asarray(gamma, dtype=np.float32).reshape(F, 1),
                "beta": np.asarray(beta, dtype=np.float32).reshape(F, 1),
                "idx16": cfg["idx16_arrs"][m],
                "colloc": cfg["colloc_arrs"][m],
                "dinv_w": cfg["dinv_arrs"][m],
                "sqd_w": cfg["sqd_arrs"][m],
                "nd1_w": cfg["nd1_arrs"][m],
                "nd2_w": cfg["nd2_arrs"][m],
            }
        )
    return maps


def assemble(cfg, results):
    M, shard = cfg["M"], cfg["shard"]
    out = np.empty((M * shard, F), dtype=np.float32)
    for m in range(M):
        out[m * shard:(m + 1) * shard] = results[m]["out_t"].T
    return out


def kernel(x, edge_idx, W, b, gamma, beta):
    from concourse.bass_utils import run_bass_kernel_spmd

    M = 8
    N = x.shape[0]
    K = W.shape[0]
    cfg = plan(np.asarray(edge_idx), N, M, nch=5)
    nc = bacc.Bacc("TRN2", num_devices=M)
    build(nc, cfg, K)
    nc.compile()
    in_maps = make_in_maps(cfg, x, W, gamma, beta)
    res = run_bass_kernel_spmd(nc, in_maps, core_ids=list(range(M)))
    return assemble(cfg, res.results)


# revision 15
# speedup vs baseline: 1.0334x; 1.0334x over previous
# ChebConv (K=4) + BatchNorm + LeakyReLU, distributed over 8 TRN2 NeuronCores.
#
# s-space formulation: track s_k = dinv * T_k, so the gather tables need no
# per-round rescale and the recurrence is
#   s_1 = nd1 * seg,   s_k = nd2 * seg - s_{k-2},   nd1 = -dinv^2, nd2 = -2dinv^2
# with seg[dst] = sum_{e->dst} s_{k-1}[src]. dinv/sqrt(deg)/nd1/nd2 come from
# the host (no on-device degree pass). Projection applies T_k = sqrt(deg)*s_k
# per window (ACT), transposes on the PE and accumulates outT += W_k^T T_k^T.
#
# Precision/speed: tables are stored as fp16 hi/lo pairs interleaved per row
# ([hi(256B) | lo(256B)] = 512B rows, same bytes as fp32, ~2^-22 relative
# accuracy). One dma_gather per edge fetches both halves; the segment sum is
# two fp16 matmuls (one-hot S is exact in fp16) accumulating into the same
# fp32 PSUM, so the PE runs at 16-bit speed with fp32-class accuracy. All
# remaining state (A/B, outT, psum, post-scales) is fp32, matching the fp32
# baseline's error profile.
#
# Nodes split into 8 shards; edges partitioned by destination core, grouped by
# (src chunk, dst window of 128); per-edge source rows gathered from per-chunk
# AllGathered tables; BN stats via a tiny AllReduce; bias b cancels in BN.

import numpy as np

from concourse import bass, bacc, mybir
import concourse.tile as tile
from concourse.masks import make_identity
from concourse.library_config import mlp as mlp_lib

P = 128
F = 128
FP32 = mybir.dt.float32
FP16 = mybir.dt.float16
I16 = mybir.dt.int16
I32 = mybir.dt.int32
AOp = mybir.AluOpType
AF = mybir.ActivationFunctionType
AX = mybir.AxisListType
BN_EPS = 1e-5
LEAKY = 0.01
MAXG = 1024  # dma_gather breaks above 1024 idxs per instruction


def _cdiv(a, b):
    return -(-a // b)


def plan(edge_idx, N, M, nch=5):
    """Host-side layout prep: edge partitioning/sorting + packed index arrays."""
    row = np.asarray(edge_idx[0], dtype=np.int64)
    col = np.asarray(edge_idx[1], dtype=np.int64)
    shard = N // M
    assert shard * M == N
    W = _cdiv(shard, P)
    win_rows = [min(P, shard - w * P) for w in range(W)]

    deg = np.bincount(row, minlength=N).astype(np.float64)
    assert deg.min() > 0, "deg-0 nodes unsupported by s-space kernel"
    dinv = 1.0 / np.sqrt(deg)
    sqrtdeg = np.sqrt(deg)

    # window counts per chunk in multiples of 4 (so projection quads never
    # span chunks); last chunk takes the remainder
    base4 = 4 * _cdiv(W, 4 * nch)
    ch_nw = []
    done = 0
    for c in range(nch):
        nw = min(base4, W - done)
        ch_nw.append(nw)
        done += nw
    assert done == W and all(nw > 0 for nw in ch_nw), ch_nw
    ch_w0 = np.cumsum([0] + ch_nw)[:-1].tolist()
    ch_r0 = [min(w0 * P, shard) for w0 in ch_w0]
    ch_rows = []
    for c in range(nch):
        r1 = min((ch_w0[c] + ch_nw[c]) * P, shard)
        ch_rows.append(r1 - ch_r0[c])
    assert all(M * r <= 32767 for r in ch_rows), (M, ch_rows)

    dst_core = col // shard
    dloc = col % shard
    win = dloc // P
    src_core = row // shard
    sloc = row % shard
    ch_bounds = np.array(ch_r0 + [shard], dtype=np.int64)
    src_ch = np.searchsorted(ch_bounds, sloc, side="right") - 1
    idx16 = src_core * np.asarray(ch_rows, dtype=np.int64)[src_ch] + (
        sloc - ch_bounds[src_ch]
    )

    # groups keyed (dst_core, src_ch, win)
    gkey = (dst_core * nch + src_ch) * W + win
    counts = np.bincount(gkey, minlength=M * nch * W).reshape(M, nch, W)
    caps = np.zeros((nch, W), dtype=np.int64)
    for c in range(nch):
        for w in range(W):
            mx = counts[:, c, w].max()
            caps[c][w] = _cdiv(mx, P) * P if mx > 0 else 0

    off_pad = np.zeros((nch, W), dtype=np.int64)
    t = 0
    for c in range(nch):
        for w in range(W):
            off_pad[c][w] = t
            t += caps[c][w]
    tot_pad = t

    order = np.lexsort((gkey, dst_core))
    idx16_arrs, colloc_arrs = [], []
    dinv_arrs, sqd_arrs, nd1_arrs, nd2_arrs = [], [], [], []
    col_in_win = dloc % P
    for m in range(M):
        sel = order[dst_core[order] == m]
        k = gkey[sel] % (nch * W)  # (c, w) flat index
        ks = np.argsort(k, kind="stable")
        sel = sel[ks]
        k = k[ks]
        grp_start = np.searchsorted(k, np.arange(nch * W))
        j = np.arange(sel.size) - grp_start[k]
        pos = off_pad.reshape(-1)[k] + j
        idx_flat = np.zeros(tot_pad, dtype=np.int16)  # pad idx 0 = valid row
        cl_flat = np.full(tot_pad, -1.0, dtype=np.float32)
        idx_flat[pos] = idx16[sel].astype(np.int16)
        cl_flat[pos] = col_in_win[sel].astype(np.float32)
        ia = np.zeros((16, tot_pad // 16), dtype=np.int16)
        ia[pos % 16, pos // 16] = idx_flat[pos]
        idx16_arrs.append(np.tile(ia, (8, 1)))
        ca = np.full((P, tot_pad // P), -1.0, dtype=np.float32)
        ca[pos % P, pos // P] = cl_flat[pos]
        colloc_arrs.append(ca)

        ids = np.arange(shard)
        gl = dinv[m * shard + ids]
        sq = sqrtdeg[m * shard + ids]

        def fold(v):
            a = np.zeros((P, W), dtype=np.float32)
            a[ids % P, ids // P] = v
            return a

        dinv_arrs.append(fold(gl))
        sqd_arrs.append(fold(sq))
        nd1_arrs.append(fold(-gl * gl))
        nd2_arrs.append(fold(-2.0 * gl * gl))

    return dict(
        N=N, M=M, shard=shard, W=W, win_rows=win_rows, nch=nch,
        ch_nw=ch_nw, ch_w0=ch_w0, ch_r0=ch_r0, ch_rows=ch_rows,
        caps=caps, off_pad=off_pad, tot_pad=tot_pad,
        idx16_arrs=idx16_arrs, colloc_arrs=colloc_arrs,
        dinv_arrs=dinv_arrs, sqd_arrs=sqd_arrs,
        nd1_arrs=nd1_arrs, nd2_arrs=nd2_arrs,
    )


def which_chunk(ch_w0, ch_nw, w):
    for c in range(len(ch_w0)):
        if ch_w0[c] <= w < ch_w0[c] + ch_nw[c]:
            return c
    raise AssertionError


def build(nc, cfg, K, no_cc=False):
    M, shard, W, nch = cfg["M"], cfg["shard"], cfg["W"], cfg["nch"]
    win_rows, caps, off_pad = cfg["win_rows"], cfg["caps"], cfg["off_pad"]
    ch_nw, ch_w0, ch_r0, ch_rows = (
        cfg["ch_nw"], cfg["ch_w0"], cfg["ch_r0"], cfg["ch_rows"],
    )
    N = cfg["N"]
    rg = [list(range(M))]
    shared_as = "Shared" if M > 4 else "Local"
    capmax = int(max(caps.max(), 1))
    F2 = 2 * F

    x_d = nc.dram_tensor("x_sh", [shard, F], FP32, kind="ExternalInput").ap()
    w_d = nc.dram_tensor("w_all", [K, F, F], FP32, kind="ExternalInput").ap()
    gam_d = nc.dram_tensor("gamma", [F, 1], FP32, kind="ExternalInput").ap()
    bet_d = nc.dram_tensor("beta", [F, 1], FP32, kind="ExternalInput").ap()
    idx_d = nc.dram_tensor(
        "idx16", [P, cfg["tot_pad"] // 16], I16, kind="ExternalInput"
    ).ap()
    cl_d = nc.dram_tensor(
        "colloc", [P, cfg["tot_pad"] // P], FP32, kind="ExternalInput"
    ).ap()
    nd_d = nc.dram_tensor("dinv_w", [P, W], FP32, kind="ExternalInput").ap()
    sq_d = nc.dram_tensor("sqd_w", [P, W], FP32, kind="ExternalInput").ap()
    n1_d = nc.dram_tensor("nd1_w", [P, W], FP32, kind="ExternalInput").ap()
    n2_d = nc.dram_tensor("nd2_w", [P, W], FP32, kind="ExternalInput").ap()
    out_d = nc.dram_tensor("out_t", [P, shard], FP32, kind="ExternalOutput").ap()

    with tile.TileContext(nc) as tc:
        with (
            tc.tile_pool(name="persist", bufs=1) as pp,
            tc.tile_pool(name="stage", bufs=3) as sp,
            tc.tile_pool(name="idxp", bufs=2) as ip,
            tc.tile_pool(name="clp", bufs=3) as cp,
            tc.tile_pool(name="hilo", bufs=2) as hp,
            tc.tile_pool(name="gath", bufs=4) as gp,
            tc.tile_pool(name="sbuild", bufs=8) as sbp,
            tc.tile_pool(name="vec", bufs=4) as vp,
            tc.tile_pool(name="roll", bufs=2) as rp,
            tc.tile_pool(name="ps_g", bufs=4, space="PSUM") as pg,
            tc.tile_pool(name="ps_sm", bufs=2, space="PSUM") as psm,
            tc.tile_pool(name="ps_o", bufs=2, space="PSUM") as po,
            tc.tile_pool(name="dram", bufs=1, space="DRAM") as dp,
        ):
            # ---- persistent SBUF
            A = pp.tile([P, W * F], FP32, name="Abuf")
            B = pp.tile([P, W * F], FP32, name="Bbuf")
            outT = pp.tile([P, shard], FP32, name="outT")
            ident = pp.tile([P, P], FP32, name="ident")
            iota_i = pp.tile([P, P], I32, name="iota_i")
            iota_f = pp.tile([P, P], FP32, name="iota_f")
            W_sb = pp.tile([P, K * F], FP32, name="W_sb")
            gam = pp.tile([P, 1], FP32, name="gam")
            bet = pp.tile([P, 1], FP32, name="bet")
            dinv = pp.tile([P, W], FP32, name="dinv")
            sqd = pp.tile([P, W], FP32, name="sqd")
            nd1 = pp.tile([P, W], FP32, name="nd1")
            nd2 = pp.tile([P, W], FP32, name="nd2")
            eps_t = pp.tile([P, 1], FP32, name="eps_t")

            make_identity(nc, ident[:])
            nc.gpsimd.iota(iota_i[:], pattern=[[1, P]], base=0, channel_multiplier=0)
            nc.gpsimd.load_library(mlp_lib)
            nc.vector.tensor_copy(iota_f[:], iota_i[:])
            nc.vector.memset(eps_t[:], BN_EPS)
            for k in range(K):
                nc.sync.dma_start(W_sb[:, k * F:(k + 1) * F], w_d[k])
            nc.sync.dma_start(gam[:], gam_d[:])
            nc.sync.dma_start(bet[:], bet_d[:])
            nc.sync.dma_start(dinv[:], nd_d[:])
            nc.sync.dma_start(sqd[:], sq_d[:])
            nc.sync.dma_start(nd1[:], n1_d[:])
            nc.sync.dma_start(nd2[:], n2_d[:])

            cap_regs = {}

            def cap_reg(cap):
                if cap not in cap_regs:
                    cap_regs[cap] = nc.gpsimd.to_reg(cap)
                return cap_regs[cap]

            def wslice(buf, w):
                return buf[:, w * F:(w + 1) * F]

            u_in = [
                dp.tile([ch_rows[c], F2], FP16, name=f"u_in{c}")
                for c in range(nch)
            ]
            u_out = [
                [
                    dp.tile(
                        [M * ch_rows[c], F2], FP16,
                        name=f"u_out{c}_{kr}", addr_space=shared_as,
                    )
                    for kr in range(K - 1)
                ]
                for c in range(nch)
            ]
            bn_in = dp.tile([P, 2], FP32, name="bn_in")
            bn_out = dp.tile([P, 2], FP32, name="bn_out", addr_space=shared_as)

            # ---- W_k projection: outT += W_k^T @ (sqd * s_k)^T per 4-window
            def wk_chain(src_buf, k, w, troll_box):
                q, pos = w // 4, w % 4
                if pos == 0:
                    troll_box[0] = rp.tile([P, 4 * P], FP32, name="troll")
                troll = troll_box[0]
                tk = sp.tile([P, F], FP32, name="tk")
                nc.scalar.activation(
                    tk[:], wslice(src_buf, w), AF.Identity,
                    scale=sqd[:, w:w + 1],
                )
                ps_t = psm.tile([P, P], FP32, name="ps_small", tag="ps_small")
                nc.tensor.transpose(ps_t[:], tk[:], ident[:])
                rw = win_rows[w]
                nc.scalar.copy(troll[:, pos * P: pos * P + rw], ps_t[:, :rw])
                if w == min(4 * q + 3, W - 1):
                    node0 = q * 4 * P
                    ncols = min(4 * P, shard - node0)
                    ps_o = po.tile([P, 4 * P], FP32, name="ps_o")
                    nc.tensor.matmul(
                        ps_o[:, :ncols],
                        lhsT=W_sb[:, k * F:(k + 1) * F],
                        rhs=troll[:, :ncols],
                        start=True, stop=True,
                    )
                    sl = outT[:, node0: node0 + ncols]
                    if k == 0:
                        nc.vector.tensor_copy(sl, ps_o[:, :ncols])
                    else:
                        nc.vector.tensor_tensor(
                            sl, sl, ps_o[:, :ncols], op=AOp.add
                        )

            def u_write(src_buf, c2, kround):
                # split chunk slab into fp16 hi/lo and store interleaved rows
                nw = ch_nw[c2]
                w0 = ch_w0[c2]
                sl = src_buf[:, w0 * F:(w0 + nw) * F]
                hi = hp.tile([P, nw * F], FP16, name="hi_sl")
                lo = hp.tile([P, nw * F], FP16, name="lo_sl")
                nc.scalar.copy(hi[:], sl)
                nc.vector.tensor_tensor(lo[:], sl, hi[:], op=AOp.subtract)
                nw_full = nw if win_rows[w0 + nw - 1] == P else nw - 1
                for half, buf in ((0, hi), (1, lo)):
                    if nw_full:
                        nc.sync.dma_start(
                            u_in[c2][: nw_full * P, :].rearrange(
                                "(w p) f -> p w f", p=P
                            )[:, :, half * F:(half + 1) * F],
                            buf[:].rearrange("p (w f) -> p w f", f=F)[
                                :, :nw_full, :
                            ],
                        )
                    rem = win_rows[w0 + nw - 1]
                    if rem < P:
                        nc.sync.dma_start(
                            u_in[c2][
                                nw_full * P:, half * F:(half + 1) * F
                            ],
                            buf[:rem, (nw - 1) * F: nw * F],
                        )
                if not no_cc:
                    nc.gpsimd.collective_compute(
                        "AllGather", AOp.bypass, replica_groups=rg,
                        ins=[u_in[c2].opt()], outs=[u_out[c2][kround].opt()],
                    )

            # ========== round 0: load x, s0 = dinv*x, W0 proj, AG ==========
            troll_box = [None]
            for w in range(W):
                rw = win_rows[w]
                xt = sp.tile([P, F], FP32, name="xt")
                nc.sync.dma_start(xt[:rw, :], x_d[w * P: w * P + rw, :])
                if rw < P:
                    nc.vector.memset(wslice(A, w), 0.0)
                nc.scalar.activation(
                    wslice(A, w)[:rw, :], xt[:rw, :], AF.Identity,
                    scale=dinv[:rw, w:w + 1],
                )
                wk_chain(A, 0, w, troll_box)
                c2 = which_chunk(ch_w0, ch_nw, w)
                if w == ch_w0[c2] + ch_nw[c2] - 1:
                    u_write(A, c2, 0)

            # ========== rounds 1..K-1 ====================================
            # slab ranges per (node-chunk block c2, src chunk c)
            def blk_range(c2, c):
                wlo, whi = ch_w0[c2], ch_w0[c2] + ch_nw[c2]
                base = int(off_pad[c][wlo])
                if whi < W:
                    end = int(off_pad[c][whi])
                elif c + 1 < nch:
                    end = int(off_pad[c + 1][0])
                else:
                    end = cfg["tot_pad"]
                return base, end

            slab_max = max(
                blk_range(c2, c)[1] - blk_range(c2, c)[0]
                for c2 in range(nch) for c in range(nch)
            )
            sb_cnt = [0]  # round-robin DVE/ACT for S builds

            def build_S(cl_src, ot, t):
                S = sbp.tile([P, P], FP16, name="Sb")
                sb_cnt[0] += 1
                if sb_cnt[0] % 3 == 0:
                    y = sbp.tile([P, P], FP16, name="Sy")
                    nc.scalar.activation(
                        y[:], iota_f[:], AF.Abs,
                        bias=cl_src[:, ot + t: ot + t + 1], scale=-1.0,
                    )
                    nc.scalar.activation(
                        S[:], y[:], AF.Relu, bias=1.0, scale=-1.0,
                    )
                else:
                    nc.vector.tensor_scalar(
                        out=S[:], in0=iota_f[:],
                        scalar1=cl_src[:, ot + t: ot + t + 1],
                        scalar2=None, op0=AOp.is_equal,
                    )
                return S

            for k in range(1, K):
                dst = B if k % 2 == 1 else A
                nd = nd1 if k == 1 else nd2
                troll_box = [None]
                for c2 in range(nch):
                    wlo, whi = ch_w0[c2], ch_w0[c2] + ch_nw[c2]
                    pend = [None]

                    def flush():
                        if pend[0] is not None:
                            pend[0]()
                            pend[0] = None

                    for c in range(nch):
                        tab = u_out[c][k - 1]
                        base, end = blk_range(c2, c)
                        it_s = ip.tile([P, slab_max // 16], I16, name="it_s")
                        cl_s = cp.tile([P, slab_max // P], FP32, name="cl_s")
                        nc.sync.dma_start(
                            it_s[:, : (end - base) // 16],
                            idx_d[:, base // 16: end // 16],
                        )
                        nc.sync.dma_start(
                            cl_s[:, : (end - base) // P],
                            cl_d[:, base // P: end // P],
                        )
                        last_c = c == nch - 1
                        for w in range(wlo, whi):
                            cap = int(caps[c][w])
                            first = all(caps[cc][w] == 0 for cc in range(c))
                            upd = None
                            if cap > 0:
                                g = cap // P
                                o16 = (int(off_pad[c][w]) - base) // 16
                                ot = (int(off_pad[c][w]) - base) // P
                                U = gp.tile([P, capmax * 2], FP16, name="Ug")
                                Uv = U[:].rearrange("p (g f) -> p g f", f=F2)
                                for goff in range(0, cap, MAXG):
                                    sub = min(MAXG, cap - goff)
                                    nc.gpsimd.dma_gather(
                                        out_ap=Uv[
                                            :, goff // P:(goff + sub) // P, :
                                        ],
                                        in_ap=tab[:],
                                        idxs_ap=it_s[
                                            :,
                                            o16 + goff // 16:
                                            o16 + (goff + sub) // 16,
                                        ],
                                        num_idxs=sub,
                                        num_idxs_reg=cap_reg(sub),
                                        elem_size=F2,
                                    )
                                ps_g = pg.tile([P, P], FP32, name="ps_g")
                                for t in range(g):
                                    S = build_S(cl_s, ot, t)
                                    nc.tensor.matmul(
                                        ps_g[:], lhsT=S[:], rhs=Uv[:, t, 0:F],
                                        start=(t == 0), stop=False,
                                    )
                                    nc.tensor.matmul(
                                        ps_g[:], lhsT=S[:], rhs=Uv[:, t, F:F2],
                                        start=False, stop=(t == g - 1),
                                    )

                                def upd(w=w, ps_g=ps_g, first=first):
                                    dw = wslice(dst, w)
                                    if first and k == 1:
                                        nc.vector.tensor_scalar_mul(
                                            out=dw, in0=ps_g[:],
                                            scalar1=nd[:, w:w + 1],
                                        )
                                    else:
                                        nc.vector.scalar_tensor_tensor(
                                            out=dw, in0=ps_g[:],
                                            scalar=nd[:, w:w + 1],
                                            in1=dw, op0=AOp.mult,
                                            op1=(
                                                AOp.subtract if first
                                                else AOp.add
                                            ),
                                        )
                            elif first and last_c and all(
                                caps[cc][w] == 0 for cc in range(nch)
                            ):

                                def upd(w=w):
                                    dw = wslice(dst, w)
                                    if k == 1:
                                        nc.vector.memset(dw, 0.0)
                                    else:
                                        nc.vector.tensor_scalar_mul(
                                            out=dw, in0=dw, scalar1=-1.0
                                        )
                            if upd is not None:
                                if last_c:

                                    def task(w=w, upd=upd):
                                        upd()
                                        wk_chain(dst, k, w, troll_box)
                                else:
                                    task = upd
                                flush()
                                pend[0] = task
                        if last_c:
                            flush()
                            if k < K - 1:
                                u_write(dst, c2, k)
                        # non-final sub-sweeps: leave the last update pending
                        # into the next sub-sweep for pipelining
                    flush()

            # ========== BatchNorm + LeakyReLU ============================
            s1 = vp.tile([P, 1], FP32, name="s1")
            nc.vector.reduce_sum(out=s1[:], in_=outT[:, :shard], axis=AX.X)
            s2 = pp.tile([P, 1], FP32, name="s2acc")
            nc.vector.memset(s2[:], 0.0)
            CH = 4 * P
            for n0 in range(0, shard, CH):
                n1 = min(n0 + CH, shard)
                sqs = rp.tile([P, 4 * P], FP32, name="troll")
                s2p = vp.tile([P, 1], FP32, name="s2p")
                nc.scalar.activation(
                    sqs[:, : n1 - n0], outT[:, n0:n1], AF.Square,
                    accum_out=s2p[:],
                )
                nc.vector.tensor_tensor(s2[:], s2[:], s2p[:], op=AOp.add)
            bn_sb = pp.tile([P, 2], FP32, name="bn_sb")
            nc.vector.tensor_copy(bn_sb[:, 0:1], s1[:])
            nc.vector.tensor_copy(bn_sb[:, 1:2], s2[:])
            nc.sync.dma_start(bn_in[:], bn_sb[:])
            if not no_cc:
                nc.gpsimd.collective_compute(
                    "AllReduce", AOp.add, replica_groups=rg,
                    ins=[bn_in.opt()], outs=[bn_out.opt()],
                )
            bnr = pp.tile([P, 2], FP32, name="bnr")
            nc.sync.dma_start(bnr[:], bn_out[:])
            mean = vp.tile([P, 1], FP32, name="mean")
            msq = vp.tile([P, 1], FP32, name="msq")
            nc.scalar.mul(mean[:], bnr[:, 0:1], 1.0 / N)
            nc.scalar.mul(msq[:], bnr[:, 1:2], 1.0 / N)
            m2 = vp.tile([P, 1], FP32, name="m2")
            var = vp.tile([P, 1], FP32, name="var")
            nc.vector.tensor_tensor(m2[:], mean[:], mean[:], op=AOp.mult)
            nc.vector.tensor_tensor(var[:], msq[:], m2[:], op=AOp.subtract)
            stdv = pp.tile([P, 1], FP32, name="stdv")
            rstd = pp.tile([P, 1], FP32, name="rstd")
            nc.scalar.activation(stdv[:], var[:], AF.Sqrt, bias=eps_t[:])
            nc.vector.reciprocal(rstd[:], stdv[:])
            Aaff = pp.tile([P, 1], FP32, name="Aaff")
            Baff = pp.tile([P, 1], FP32, name="Baff")
            mA = vp.tile([P, 1], FP32, name="mA")
            nc.vector.tensor_tensor(Aaff[:], gam[:], rstd[:], op=AOp.mult)
            nc.vector.tensor_tensor(mA[:], mean[:], Aaff[:], op=AOp.mult)
            nc.vector.tensor_tensor(Baff[:], bet[:], mA[:], op=AOp.subtract)
            for n0 in range(0, shard, CH):
                n1 = min(n0 + CH, shard)
                ts = rp.tile([P, 4 * P], FP32, name="troll")
                nc.scalar.activation(
                    ts[:, : n1 - n0], outT[:, n0:n1], AF.Identity,
                    bias=Baff[:], scale=Aaff[:],
                )
                nc.vector.scalar_tensor_tensor(
                    out=outT[:, n0:n1], in0=ts[:, : n1 - n0], scalar=LEAKY,
                    in1=ts[:, : n1 - n0], op0=AOp.mult, op1=AOp.max,
                )
            nc.sync.dma_start(out_d[:], outT[:, :shard])
    return nc


def make_in_maps(cfg, x, W_, gamma, beta):
    M, shard = cfg["M"], cfg["shard"]
    x = np.asarray(x, dtype=np.float32)
    maps = []
    for m in range(M):
        maps.append(
            {
                "x_sh": np.ascontiguousarray(x[m * shard:(m + 1) * shard]),
                "w_all": np.asarray(W_, dtype=np.float32),
                "gamma": np.asarray(gamma, dtype=np.float32).reshape(F, 1),
                "beta": np.asarray(beta, dtype=np.float32).reshape(F, 1),
                "idx16": cfg["idx16_arrs"][m],
                "colloc": cfg["colloc_arrs"][m],
                "dinv_w": cfg["dinv_arrs"][m],
                "sqd_w": cfg["sqd_arrs"][m],
                "nd1_w": cfg["nd1_arrs"][m],
                "nd2_w": cfg["nd2_arrs"][m],
            }
        )
    return maps


def assemble(cfg, results):
    M, shard = cfg["M"], cfg["shard"]
    out = np.empty((M * shard, F), dtype=np.float32)
    for m in range(M):
        out[m * shard:(m + 1) * shard] = results[m]["out_t"].T
    return out


def kernel(x, edge_idx, W, b, gamma, beta):
    from concourse.bass_utils import run_bass_kernel_spmd

    M = 8
    N = x.shape[0]
    K = W.shape[0]
    cfg = plan(np.asarray(edge_idx), N, M, nch=5)
    nc = bacc.Bacc("TRN2", num_devices=M)
    build(nc, cfg, K)
    nc.compile()
    in_maps = make_in_maps(cfg, x, W, gamma, beta)
    res = run_bass_kernel_spmd(nc, in_maps, core_ids=list(range(M)))
    return assemble(cfg, res.results)
